# revision 33
# baseline (speedup 1.0000x reference)
"""Cross-attention kernel for 8 trn2 NeuronCores.

Problem: B=2, Lq=Lk=2048, D=1024, H=16, dh=64.
  q/k/v = Linear(x); q,k L2-normalized per head; S = q@k.T * 1/8;
  key-pad mask -> -1e9; softmax; mask-aware renorm; eps-smooth toward
  uniform-over-valid; out = attn@v merged -> out_proj.

Sharding: core c handles batch b=c//4, heads [4*(c%4), 4*(c%4)+4)
(two head pairs of 2 heads). Each core computes a partial output
projection over its 256 head dims; the host sums the 4 partials per
batch and adds the output bias.

Key optimizations over the v1 baseline:
  - Host-side key compaction: ~half the keys are padding; only valid
    keys (padded to a multiple of 128) are shipped/projected/attended.
    Masked keys contribute exactly 0 to P, rowsum and AV, so dropping
    them is mathematically identical. The Bass program is built per
    runtime KT (cached).
  - Rowsum for free: v carries a 65th column = 1/0.9, so the AV matmul
    output column 64 is rowsum(P)/0.9 and the old per-kt rowsum matmuls
    disappear. AV runs in [q, d] orientation (lhsT = P tile), so the
    renorm divide + eps smoothing collapse into one scalar_tensor_tensor
    with a per-partition 0.9/rs scalar; PE transposes restore [d, q]
    for the output projection.
  - q/k linear bias folded into the Square activation and the final
    normalize stt (no bias matmuls).
  - q-projection pipelined per q-chunk inside the attention loop
    (sharing the S psum tiles), so attention starts after only
    k-proj + v-proj + one q chunk.
  - Out-projection interleaved per q-chunk (no serial tail); the
    0.1*vmean eps-smoothing term rides in the stt; host adds bo.
  - Few large DMAs into resident SBUF x tiles instead of 192 small ones.
"""

import ml_dtypes
import numpy as np

import concourse.bass as bass
from concourse import bacc
import concourse.mybir as mybir
import concourse.tile as tile
from concourse.bass_utils import run_bass_kernel_spmd

F32 = mybir.dt.float32
BF16 = mybir.dt.bfloat16
AF = mybir.ActivationFunctionType
ALU = mybir.AluOpType

B, L, D = 2, 2048, 1024
H, DH = 16, 64
HEADS_PER_CORE = 4          # -> 256 dims per core, 2 head pairs
HPC = HEADS_PER_CORE * DH   # 256
SCALE = 0.125               # 1/sqrt(64) / ATTN_TEMP
EPS_SMOOTH = 0.1
INV09 = 1.0 / (1.0 - EPS_SMOOTH)
MASK_BIAS = -30000.0
N_CORES = 8
QC = L // 512               # 4 q chunks
NCH = D // 128              # 8 contraction chunks for projections


def _build_nc(KT):
    LK = KT * 128
    # k-token chunks of <=512 for the k projection
    kchunks = []
    o = 0
    while o < LK:
        sz = min(512, LK - o)
        kchunks.append((o, sz))
        o += sz

    nc = bacc.Bacc(None)

    xqT = nc.dram_tensor("xqT", [D, L], BF16, kind="ExternalInput")
    xkT = nc.dram_tensor("xkT", [D, LK], BF16, kind="ExternalInput")
    xvT = nc.dram_tensor("xvT", [D, LK], BF16, kind="ExternalInput")
    wq_t = nc.dram_tensor("wq_t", [D, HPC], BF16, kind="ExternalInput")
    wk_t = nc.dram_tensor("wk_t", [D, HPC], BF16, kind="ExternalInput")
    wv_t = nc.dram_tensor("wv_t", [D, HPC], BF16, kind="ExternalInput")
    wo_t = nc.dram_tensor("wo_t", [HPC, D], BF16, kind="ExternalInput")
    bq = nc.dram_tensor("bq", [2, 128, 1], F32, kind="ExternalInput")
    bk = nc.dram_tensor("bk", [2, 128, 1], F32, kind="ExternalInput")
    bv = nc.dram_tensor("bv", [1, HPC], BF16, kind="ExternalInput")
    mbias = nc.dram_tensor("mbias", [128, KT], F32, kind="ExternalInput")
    vmb = nc.dram_tensor("vmb", [128, HEADS_PER_CORE, DH], F32,
                         kind="ExternalInput")
    partial = nc.dram_tensor("partial", [L, D], BF16, kind="ExternalOutput")

    with tile.TileContext(nc) as tc:
        with (
            tc.tile_pool(name="consts", bufs=1) as consts,
            tc.tile_pool(name="wpool", bufs=1) as wpool,
            tc.tile_pool(name="xres", bufs=1) as xres,
            tc.tile_pool(name="persist", bufs=1) as persist,
            tc.tile_pool(name="l2pool", bufs=4) as l2pool,
            tc.tile_pool(name="ppool", bufs=3) as ppool,
            tc.tile_pool(name="normpool", bufs=4) as normpool,
            tc.tile_pool(name="ostpool", bufs=4) as ostpool,
        ):
            # ---- constants ----
            ones_row = consts.tile([1, 512], BF16, tag="ones_row")
            nc.vector.memset(ones_row, 1.0)
            blockdiag = consts.tile([128, 128], BF16, tag="blockdiag")
            nc.vector.memset(blockdiag, 0.0)
            nc.vector.memset(blockdiag[0:64, 0:64], 1.0)
            nc.vector.memset(blockdiag[64:128, 64:128], 1.0)
            bias_sb = {}
            for name, hnd in (("q", bq), ("k", bk)):
                for hp in range(2):
                    t = consts.tile([128, 1], F32, tag=f"b{name}{hp}")
                    nc.sync.dma_start(out=t, in_=hnd[hp])
                    bias_sb[(name, hp)] = t

            # ---- weights + x in dependency order (k gates attention) ----
            w_sb = {}
            xq_sb = xres.tile([128, NCH, L], BF16, tag="xq")
            xk_sb = xres.tile([128, NCH, LK], BF16, tag="xk")
            xv_sb = xres.tile([128, NCH, LK], BF16, tag="xv")

            def load_w(name, hnd):
                t = wpool.tile([128, NCH, HPC], BF16, tag=f"w{name}")
                nc.sync.dma_start(
                    out=t, in_=hnd.rearrange("(c p) m -> p c m", p=128)
                )
                w_sb[name] = t

            load_w("k", wk_t)
            for c in range(NCH):
                nc.sync.dma_start(
                    out=xk_sb[:, c, :], in_=xkT[c * 128:(c + 1) * 128, :]
                )
            load_w("v", wv_t)
            for c in range(NCH):
                nc.sync.dma_start(
                    out=xv_sb[:, c, :], in_=xvT[c * 128:(c + 1) * 128, :]
                )
            load_w("q", wq_t)
            for c in range(NCH):
                nc.sync.dma_start(
                    out=xq_sb[:, c, :], in_=xqT[c * 128:(c + 1) * 128, :]
                )
            wo_sb = wpool.tile([128, 2, D], BF16, tag="wo")
            nc.sync.dma_start(
                out=wo_sb, in_=wo_t.rearrange("(h p) m -> p h m", p=128)
            )
            mbias_sb = consts.tile([128, KT], F32, tag="mbias")
            nc.sync.dma_start(out=mbias_sb, in_=mbias[:, :])
            vmb_sb = consts.tile([128, HEADS_PER_CORE, DH], F32, tag="vmb")
            nc.sync.dma_start(out=vmb_sb, in_=vmb[:, :, :])
            bv_sb = consts.tile([1, HPC], BF16, tag="bv")
            nc.sync.dma_start(out=bv_sb, in_=bv[:, :])

            # ---- persistent activations ----
            qTn = [persist.tile([128, L], BF16, tag=f"qTn{hp}", name=f"qTn{hp}")
                   for hp in range(2)]
            kTn = [persist.tile([128, LK], BF16, tag=f"kTn{hp}",
                                name=f"kTn{hp}") for hp in range(2)]
            # v with a 65th column of 1/0.9 per head -> AV col 64 = rs/0.9
            v65 = persist.tile([128, KT, HEADS_PER_CORE, DH + 1], BF16,
                               tag="v65")
            nc.vector.memset(v65[:, :, :, DH:DH + 1], INV09)

            def proj_qk_chunk(pool, name, xsb, dst, t0, tsz):
                """One token chunk of the q/k projection + L2 norm.

                Uses two [128, 1024] psum tiles from `pool` (halves = the
                two head pairs): one for the projection, one for the
                squared-norm blockdiag matmuls. The linear bias rides in
                the Square activation and the final normalize stt.
                """
                pp = pool.tile([128, 1024], F32, tag="s", name=f"pp_{name}")
                n2 = pool.tile([128, 1024], F32, tag="s", name=f"n2_{name}")
                for hp in range(2):
                    hsl = slice(hp * 512, hp * 512 + tsz)
                    for c in range(NCH):
                        nc.tensor.matmul(
                            pp[:, hsl],
                            lhsT=w_sb[name][:, c, hp * 128:(hp + 1) * 128],
                            rhs=xsb[:, c, t0:t0 + tsz],
                            start=(c == 0),
                            stop=(c == NCH - 1),
                        )
                for hp in range(2):
                    hsl = slice(hp * 512, hp * 512 + tsz)
                    bias_col = bias_sb[(name, hp)]
                    # L2 norm over each head's 64 dims of (q + b)
                    sq = l2pool.tile([128, 512], BF16, tag="sq")
                    nc.scalar.activation(
                        sq[:, 0:tsz], pp[:, hsl], AF.Square, bias=bias_col
                    )
                    nc.tensor.matmul(
                        n2[:, hsl], lhsT=blockdiag,
                        rhs=sq[:, 0:tsz], start=True, stop=True,
                    )
                    nlen = l2pool.tile([128, 512], F32, tag="nlen")
                    nc.scalar.activation(nlen[:, 0:tsz], n2[:, hsl], AF.Sqrt)
                    rnorm = l2pool.tile([128, 512], F32, tag="rnorm")
                    nc.vector.reciprocal_approx_fast(
                        rnorm[:, 0:tsz], nlen[:, 0:tsz]
                    )
                    # (q + b) * 1/|q + b|
                    nc.vector.scalar_tensor_tensor(
                        out=dst[hp][:, t0:t0 + tsz],
                        in0=pp[:, hsl],
                        scalar=bias_col,
                        in1=rnorm[:, 0:tsz],
                        op0=ALU.add,
                        op1=ALU.mult,
                    )

            # ---- prologue: k-proj, v-proj, q-proj(chunk 0) ----
            with (
                tc.tile_pool(name="ps_proj", bufs=3, space="PSUM") as ps_proj,
                tc.tile_pool(name="ps_v", bufs=2, space="PSUM") as ps_v,
            ):
                for t0, tsz in kchunks:
                    proj_qk_chunk(ps_proj, "k", xk_sb, kTn, t0, tsz)

                # v: v[t, dout] = sum_c xT[c][:, t].T @ w[c]
                for tt in range(KT):
                    vp = ps_v.tile([128, HPC], F32, tag="vproj")
                    for c in range(NCH):
                        nc.tensor.matmul(
                            vp,
                            lhsT=xv_sb[:, c, tt * 128:(tt + 1) * 128],
                            rhs=w_sb["v"][:, c, :],
                            start=(c == 0), stop=False,
                        )
                    nc.tensor.matmul(
                        vp, lhsT=ones_row[:, 0:128], rhs=bv_sb,
                        start=False, stop=True,
                    )
                    nc.vector.tensor_copy(v65[:, tt, :, 0:DH], vp)

                for i in range(QC):
                    proj_qk_chunk(ps_proj, "q", xq_sb, qTn, i * 512, 512)

            # ---- attention + pipelined q-proj + out-projection ----
            with (
                tc.tile_pool(name="ps_S", bufs=2, space="PSUM") as ps_S,
                tc.tile_pool(name="ps_O", bufs=1, space="PSUM") as ps_O,
                tc.tile_pool(name="ps_out", bufs=2, space="PSUM") as ps_out,
                tc.tile_pool(name="ofpool", bufs=2) as ofpool,
            ):
                for qc in range(QC):
                    qsl = slice(qc * 512, (qc + 1) * 512)
                    ofin = [
                        ofpool.tile([128, 512], BF16, tag=f"ofin{hp}",
                                    name=f"ofin{hp}")
                        for hp in range(2)
                    ]
                    for hp in range(2):
                        o_ps = [
                            ps_O.tile([128, 4, DH + 1], F32, tag=f"o{h}",
                                      name=f"o{h}")
                            for h in range(2)
                        ]
                        # 8 accumulation regions share these banks at
                        # different free offsets; a start=True reset would
                        # wipe sibling regions, so zero once + accumulate.
                        for h in range(2):
                            nc.vector.memset(o_ps[h], 0.0)
                        for kt in range(KT):
                            s_ps = ps_S.tile([128, 1024], F32, tag="s")
                            ksl = slice(kt * 128, (kt + 1) * 128)
                            # S_T = k̂.T q̂ per head, row-packed (K=64 each)
                            nc.tensor.matmul(
                                s_ps[:, 0:512],
                                lhsT=kTn[hp][0:64, ksl],
                                rhs=qTn[hp][0:64, qsl],
                                start=True, stop=True,
                            )
                            nc.tensor.matmul(
                                s_ps[:, 512:1024],
                                lhsT=kTn[hp][64:128, ksl],
                                rhs=qTn[hp][64:128, qsl],
                                start=True, stop=True,
                            )
                            # P = exp(SCALE*S + pad_bias); padded keys -> 0
                            p_sb = ppool.tile([128, 1024], BF16, tag="p")
                            nc.scalar.activation(
                                p_sb, s_ps, AF.Exp,
                                bias=mbias_sb[:, kt:kt + 1], scale=SCALE,
                            )
                            # O[q, d(+rs)] += P_tile.T @ [v | 1/0.9]
                            for h in range(2):
                                for qi in range(4):
                                    nc.tensor.matmul(
                                        o_ps[h][:, qi, :],
                                        lhsT=p_sb[:, h * 512 + qi * 128:
                                                  h * 512 + (qi + 1) * 128],
                                        rhs=v65[:, kt, hp * 2 + h, :],
                                        start=False, stop=(kt == KT - 1),
                                        skip_group_check=True,
                                    )
                        # renorm + eps smoothing into a head-paired [q, d]
                        # tile, then one [128,128] transpose per q subtile
                        oT = normpool.tile([128, 4, 128], BF16, tag="oT")
                        for h in range(2):
                            rr = normpool.tile([128, 4], F32, tag="rr")
                            nc.vector.reciprocal_approx_fast(
                                rr, o_ps[h][:, :, DH:DH + 1]
                            )
                            for qi in range(4):
                                # (o * 0.9/rs) + 0.1*vmean
                                nc.vector.scalar_tensor_tensor(
                                    out=oT[:, qi, h * DH:(h + 1) * DH],
                                    in0=o_ps[h][:, qi, 0:DH],
                                    scalar=rr[:, qi:qi + 1],
                                    in1=vmb_sb[:, hp * 2 + h, :],
                                    op0=ALU.mult,
                                    op1=ALU.add,
                                )
                        for qi in range(4):
                            # [q, d] -> [d, q] via XBAR DMA transpose
                            nc.sync.dma_start(
                                out=ofin[hp][:, qi * 128:(qi + 1) * 128],
                                in_=oT[:, qi, :],
                                transpose=True,
                            )
                            if hp == 1:
                                # out projection for this token subtile
                                # (both head pairs now final)
                                tsl = slice(qc * 512 + qi * 128,
                                            qc * 512 + (qi + 1) * 128)
                                for nh in range(2):
                                    nsl = slice(nh * 512, (nh + 1) * 512)
                                    op = ps_out.tile([128, 512], F32,
                                                     tag="oproj")
                                    nc.tensor.matmul(
                                        op,
                                        lhsT=ofin[0][:, qi * 128:
                                                     (qi + 1) * 128],
                                        rhs=wo_sb[:, 0, nsl],
                                        start=True, stop=False,
                                    )
                                    nc.tensor.matmul(
                                        op,
                                        lhsT=ofin[1][:, qi * 128:
                                                     (qi + 1) * 128],
                                        rhs=wo_sb[:, 1, nsl],
                                        start=False, stop=True,
                                    )
                                    ost = ostpool.tile([128, 512], BF16,
                                                       tag="ost")
                                    nc.vector.tensor_copy(ost, op)
                                    # store on the idle gpsimd software
                                    # DGE; sync is busy with transposes
                                    nc.gpsimd.dma_start(
                                        out=partial[tsl, nsl], in_=ost
                                    )

    nc.finalize()
    return nc


_NC_CACHE = {}


def _get_nc(KT):
    if KT not in _NC_CACHE:
        _NC_CACHE[KT] = _build_nc(KT)
    return _NC_CACHE[KT]


def kernel(q_in, k_in, v_in, kv_pad_mask, Wq, bq, Wk, bk, Wv, bv, Wo, bo,
           _trace=False):
    f32 = np.float32
    bf = ml_dtypes.bfloat16
    q_in = np.asarray(q_in, f32)
    k_in = np.asarray(k_in, f32)
    v_in = np.asarray(v_in, f32)
    mask = np.asarray(kv_pad_mask, bool)
    Wq, bq, Wk, bk, Wv, bv, Wo, bo = (
        np.asarray(a, f32) for a in (Wq, bq, Wk, bk, Wv, bv, Wo, bo)
    )

    # host-side compaction of valid keys, padded to a multiple of 128
    valid_idx = [np.nonzero(~mask[b])[0] for b in range(B)]
    nv = [len(ix) for ix in valid_idx]
    KT = max(1, max((n + 127) // 128 for n in nv))
    LK = KT * 128

    nc = _get_nc(KT)

    xT = {}
    mb = {}
    for b in range(B):
        ix = valid_idx[b]
        kc = np.zeros((LK, D), f32)
        vc = np.zeros((LK, D), f32)
        kc[:nv[b]] = k_in[b][ix]
        vc[:nv[b]] = v_in[b][ix]
        xT[("q", b)] = np.ascontiguousarray(q_in[b].T).astype(bf)
        xT[("k", b)] = np.ascontiguousarray(kc.T).astype(bf)
        xT[("v", b)] = np.ascontiguousarray(vc.T).astype(bf)
        mrow = np.full(LK, MASK_BIAS, f32)
        mrow[:nv[b]] = 0.0
        mb[b] = np.ascontiguousarray(mrow.reshape(KT, 128).T)

    in_maps = []
    for core in range(N_CORES):
        b = core // 4
        h0 = (core % 4) * HEADS_PER_CORE
        rows = slice(h0 * DH, h0 * DH + HPC)
        # 0.1 * mean_over_valid(v) for this core's 256 dims, replicated
        # across partitions for the stt broadcast operand
        vm = (v_in[b][valid_idx[b]].mean(axis=0) if nv[b] else
              np.zeros(D, f32))
        vm = (vm @ Wv[rows].T + bv[rows]) * EPS_SMOOTH
        vmb = np.broadcast_to(
            vm.reshape(HEADS_PER_CORE, DH), (128, HEADS_PER_CORE, DH)
        )
        in_maps.append({
            "xqT": xT[("q", b)],
            "xkT": xT[("k", b)],
            "xvT": xT[("v", b)],
            "wq_t": np.ascontiguousarray(Wq[rows].T).astype(bf),
            "wk_t": np.ascontiguousarray(Wk[rows].T).astype(bf),
            "wv_t": np.ascontiguousarray(Wv[rows].T).astype(bf),
            "wo_t": np.ascontiguousarray(Wo[:, rows].T).astype(bf),
            "bq": np.ascontiguousarray(bq[rows].reshape(2, 128, 1)),
            "bk": np.ascontiguousarray(bk[rows].reshape(2, 128, 1)),
            "bv": np.ascontiguousarray(bv[rows].reshape(1, HPC)).astype(bf),
            "mbias": mb[b],
            "vmb": np.ascontiguousarray(vmb.astype(f32)),
        })

    res = run_bass_kernel_spmd(nc, in_maps, core_ids=list(range(N_CORES)),
                               trace=_trace)
    out = np.zeros((B, L, D), f32)
    for core in range(N_CORES):
        out[core // 4] += res.results[core]["partial"]
    out += bo[None, None, :]
    if _trace:
        kernel._last_result = res
    return out
